# revision 21
# baseline (speedup 1.0000x reference)
"""Trainium2 Bass kernel for nn_BoothGroupQuant.

Booth/NAF group quantization: q = rne(x*128); NAF-decompose each q into
signed power-of-two digits; per group of 16 consecutive elements keep only
the 8 largest-exponent digits (ties: lower exponent first, then element
order); reconstruct and scale by 1/128.

Core identity: with t = 3q, u = t ^ q, the NAF nonzero-digit mask of q is u
(digit at exponent e <-> bit e+1), positive digits at u & t, negative at
u & q -- valid directly on two's-complement negatives.  Per-group top-8
selection via int16 SWAR band counters (4 bands of 3 exponents), two
grouped reduces, one segmented scan for in-band ranks, and a packed
guard-bit compare.  Design range |q| <= 2730 (actual data max 668).

Engine split: DVE does bitwise SWAR; ScalarE (ACT) does pure-arithmetic
ops; sharded over 8 cores on the flat element axis.
"""
import os
import sys

import numpy as np

for _p in ("/opt/trn_rl_repo", "/root/.axon_site/_ro/trn_rl_repo"):
    if os.path.isdir(_p) and _p not in sys.path:
        sys.path.insert(0, _p)

import concourse.bacc as bacc
import concourse.mybir as mybir
from concourse import bass_utils
from concourse.tile import TileContext

N_CORES = 8
FULL_SHAPE = (4, 1024, 32, 32)
N_TOTAL = 4 * 1024 * 32 * 32          # 4194304
N_CORE = N_TOTAL // N_CORES           # 524288
P = 128                               # SBUF partitions
F_TOTAL = N_CORE // P                 # 4096 free elems per partition
CHUNKS = (512, 1536, 1536, 512)       # free elems per chunk (head/tail small)
F_CHUNK = max(CHUNKS)
SF = 0.0078125

i16 = mybir.dt.int16
f32 = mybir.dt.float32
Alu = mybir.AluOpType
Act = mybir.ActivationFunctionType
AX = mybir.AxisListType

_CACHE = {}


def _build():
    nc = bacc.Bacc("TRN2")
    x_in = nc.dram_tensor("x", [P, F_TOTAL], f32, kind="ExternalInput")
    y_out = nc.dram_tensor("y", [P, F_TOTAL], f32, kind="ExternalOutput")

    with TileContext(nc) as tc:
        with tc.tile_pool(name="const", bufs=1) as cpool:
            # segment mask: 0 at each group start, 1 elsewhere
            seg = cpool.tile([P, F_CHUNK], i16)
            nc.vector.memset(seg, 1)
            nc.vector.memset(
                seg.rearrange("p (g s) -> p g s", s=16)[:, :, 0:1], 0)

            with tc.tile_pool(name="work", bufs=3) as pool:
                off = 0
                for ci, fc in enumerate(CHUNKS):
                    _chunk(nc, pool, seg, x_in, y_out, off, fc,
                           first=(ci == 0), last=(ci == len(CHUNKS) - 1))
                    off += fc

    nc.compile()
    return nc


def _chunk(nc, pool, seg, x_in, y_out, off, fc, first=False, last=False):
    Fc = fc
    Gc = Fc // 16
    sl = slice(off, off + fc)

    def grp(ap):
        return ap.rearrange("p (g s) -> p g s", s=16)

    def bc(tiny):
        return tiny[:, :, None].broadcast_to((P, Gc, 16))

    def full(nm, dt=i16, nb=None):
        return pool.tile([P, Fc], dt, name=nm, tag=nm, bufs=nb)

    def tiny(nm, dt=i16):
        return pool.tile([P, Gc], dt, name=nm, tag=nm)

    V, S = nc.vector, nc.scalar

    xt = full("xt", f32, nb=2)
    nc.sync.dma_start(out=xt, in_=x_in[:, sl])

    # q = rne(x*128) int16   (|q| <= 668 on this input; no clip needed)
    q = full("q", nb=2)
    S.activation(q, xt, Act.Copy, scale=128.0)
    # t = 3q (ACT) ; u = t ^ q  (NAF mask, bits 1..12)
    t = full("t", nb=2)
    S.activation(t, q, Act.Copy, scale=3.0)
    u = full("u", nb=2)
    V.tensor_tensor(u, t, q, Alu.bitwise_xor)

    # band popcounts: c fields {0,3,6,9} = per-element band counts (0..3)
    A = full("A")
    V.tensor_scalar(A, u, 1, 0x249, Alu.logical_shift_right, Alu.bitwise_and)
    B = full("B")
    V.tensor_scalar(B, u, 2, 0x249, Alu.logical_shift_right, Alu.bitwise_and)
    C = full("C")
    V.tensor_scalar(C, u, 3, 0x249, Alu.logical_shift_right, Alu.bitwise_and)
    V.tensor_tensor(A, A, B, Alu.add)          # c12
    V.tensor_tensor(A, A, C, Alu.add)          # c
    D = pool.tile([P, 2 * Fc], i16, name="D", tag="D")
    V.tensor_scalar(D[:, 0:Fc], A, 0x1C7, None, Alu.bitwise_and)          # ce
    V.tensor_scalar(D[:, Fc:], A, 3, 0x1C7, Alu.logical_shift_right,
                    Alu.bitwise_and)                                      # co

    # group band totals in one reduce (fields 0-5, 6-11; sums <= 48)
    R2 = pool.tile([P, 2 * Gc], i16, name="R2", tag="R2")
    with nc.allow_low_precision(reason="exact small int sums"):
        V.tensor_reduce(R2, D.rearrange("p (k g s) -> p (k g) s", s=16, k=2),
                        AX.X, Alu.add)
    RE = R2[:, 0:Gc]
    RO = R2[:, Gc:]

    # tiny: band sums, crossing band b*, theta  (B0 = RE&63 is unused)
    B2 = tiny("B2")
    V.tensor_scalar(B2, RE, 6, None, Alu.logical_shift_right)
    B1 = tiny("B1")
    V.tensor_scalar(B1, RO, 63, None, Alu.bitwise_and)
    B3 = tiny("B3")
    V.tensor_scalar(B3, RO, 6, None, Alu.logical_shift_right)
    s2 = tiny("s2")
    V.tensor_tensor(s2, B3, B2, Alu.add)
    s1 = tiny("s1")
    V.tensor_tensor(s1, s2, B1, Alu.add)
    g3 = tiny("g3")
    V.tensor_scalar(g3, B3, 8, None, Alu.is_ge)
    g2 = tiny("g2")
    V.tensor_scalar(g2, s2, 8, None, Alu.is_ge)
    g1 = tiny("g1")
    V.tensor_scalar(g1, s1, 8, None, Alu.is_ge)
    bstar = tiny("bstar")
    V.tensor_tensor(bstar, g3, g2, Alu.add)
    V.tensor_tensor(bstar, bstar, g1, Alu.add)


    # Cab = B3*(1-g3) + B2*(1-g2) + B1*(1-g1);  theta = 8 - Cab in [1, 8]
    ng3 = tiny("ng3")
    V.tensor_scalar(ng3, g3, -1, 1, Alu.mult, Alu.add)
    ng2 = tiny("ng2")
    V.tensor_scalar(ng2, g2, -1, 1, Alu.mult, Alu.add)
    ng1 = tiny("ng1")
    V.tensor_scalar(ng1, g1, -1, 1, Alu.mult, Alu.add)
    V.tensor_tensor(ng3, B3, ng3, Alu.mult)
    V.tensor_tensor(ng2, B2, ng2, Alu.mult)
    V.tensor_tensor(ng1, B1, ng1, Alu.mult)
    V.tensor_tensor(ng3, ng3, ng2, Alu.add)
    V.tensor_tensor(ng3, ng3, ng1, Alu.add)    # = Cab
    theta = tiny("theta")
    V.tensor_scalar(theta, ng3, -1, 8, Alu.mult, Alu.add)

    # stage-2 per-element: w = u >> amt; s = spread(w & 7) at bits {0,5,10}
    amtx = full("amtx", nb=2)
    S.activation(grp(amtx), bc(bstar), Act.Copy, scale=3.0, bias=1.0)
    w = full("w")
    V.tensor_tensor(w, u, amtx, Alu.logical_shift_right)
    s = full("s")
    V.tensor_scalar(s, w, 7, None, Alu.bitwise_and)
    sm = full("sm", nb=2)
    S.activation(sm, s, Act.Copy, scale=float(0x111))
    V.tensor_scalar(s, sm, 0x421, None, Alu.bitwise_and)
    Pm = full("Pm")
    V.tensor_tensor_scan(Pm, seg[:, 0:Fc], s, 0.0, Alu.mult, Alu.add)

    # tiny: per-exp thresholds packed with guard bits (strided group-last read)
    TPv = grp(Pm)[:, :, 15]
    n2 = tiny("n2")
    V.tensor_scalar(n2, TPv, 10, 31, Alu.logical_shift_right, Alu.bitwise_and)
    n1 = tiny("n1")
    V.tensor_scalar(n1, TPv, 5, 31, Alu.logical_shift_right, Alu.bitwise_and)
    th1 = tiny("th1")
    V.tensor_tensor(th1, theta, n2, Alu.subtract)
    th0 = tiny("th0")
    V.tensor_tensor(th0, th1, n1, Alu.subtract)
    th1c = tiny("th1c")
    V.tensor_scalar(th1c, th1, 0, 32, Alu.max, Alu.mult)
    th0c = tiny("th0c")
    V.tensor_scalar(th0c, th0, 0, None, Alu.max)
    t2s = tiny("t2s")
    V.tensor_scalar(t2s, theta, 1024, None, Alu.mult)
    V.tensor_tensor(th0c, th0c, th1c, Alu.add)
    V.tensor_tensor(th0c, th0c, t2s, Alu.add)


    # per-element packed compare: guard bit j <=> excl_rank_j < theta_j
    V.tensor_tensor(Pm, Pm, s, Alu.subtract)                  # Y (excl ranks)
    thx = full("thx", nb=2)
    S.activation(grp(thx), bc(th0c), Act.Copy, bias=float(0x4210 - 0x421))
    X = full("X", nb=2)
    V.tensor_tensor(X, thx, Pm, Alu.subtract)
    # gather guard bits {4,9,14} -> band keep mask (int16-safe two-mult form)
    V.tensor_scalar(s, X, 12, 4, Alu.logical_shift_right, Alu.bitwise_and)
    # K3hi now in s
    V.tensor_scalar(Pm, X, 4, 0x21, Alu.logical_shift_right, Alu.bitwise_and)
    K3m = full("K3m", nb=2)
    S.activation(K3m, Pm, Act.Copy, scale=float(0x11))
    V.tensor_scalar(K3m, K3m, 4, 3, Alu.logical_shift_right, Alu.bitwise_and)
    V.tensor_tensor(s, s, K3m, Alu.bitwise_or)                # K3
    V.tensor_scalar(s, s, -8, None, Alu.bitwise_or)           # Kband
    V.tensor_tensor(w, w, s, Alu.bitwise_and)                 # wk
    V.tensor_tensor(w, w, amtx, Alu.logical_shift_left)   # UK

    # val = UK - 2*(UK & q)
    V.tensor_tensor(q, w, q, Alu.bitwise_and)                 # NM
    NM2 = full("NM2", nb=2)
    S.activation(NM2, q, Act.Copy, scale=2.0)
    V.tensor_tensor(w, w, NM2, Alu.subtract)                  # val

    yt = full("yt", f32, nb=2)
    if last:
        V.tensor_scalar(yt, w, SF / 2.0, None, Alu.mult)
    else:
        S.activation(yt, w, Act.Copy, scale=SF / 2.0)
    nc.sync.dma_start(out=y_out[:, sl], in_=yt)


def _get_nc():
    if "nc" not in _CACHE:
        _CACHE["nc"] = _build()
    return _CACHE["nc"]


def kernel(x: np.ndarray, _trace: bool = False, _trace_kwargs=None):
    assert x.shape == FULL_SHAPE and x.dtype == np.float32, (x.shape, x.dtype)
    nc = _get_nc()
    flat = np.ascontiguousarray(x).reshape(N_CORES, P, F_TOTAL)
    in_maps = [{"x": flat[i]} for i in range(N_CORES)]
    kw = {}
    if _trace:
        kw = {"trace": True, **(_trace_kwargs or {})}
    res = bass_utils.run_bass_kernel_spmd(
        nc, in_maps, core_ids=list(range(N_CORES)), **kw)
    out = np.stack([res.results[i]["y"] for i in range(N_CORES)], axis=0)
    out = out.reshape(FULL_SHAPE).astype(np.float32)
    if _trace:
        return out, res
    return out
